# revision 5
# baseline (speedup 1.0000x reference)
"""DeltaNet chunked delta-rule kernel for Trainium2 (Bass/Tile), 8-core SPMD.

v4: group-of-4-chunk processing — all pass-A psums are [*, 4, C] full-bank
tiles so each DVE/Act drain instruction moves 4 chunks' worth of data
(halved instruction count vs pair processing). bf16 matmuls; Q-powers via
PE transpose; q-norm folded into the final out-drain; u0 fused into the
scan PSUM accumulation; S kept in bf16; R0 fused into the level-1 R drain
(R1 = I + Q0 + Q1 + Q1 Q0 accumulated in PSUM, +I via the DVE add-drain).

Math (identical to reference for any chunk size; C=128):
  kh = l2norm(k); vb = v*beta; kb = kh*beta; q stays RAW (scale 1/|q|
  applied at the final out drain as a per-row Act scale).
  Per chunk: T = kb kh^T ; P = -tril(T,-1) ; Q = P^T ;
  inv^T = prod_j (I + Q^(2^j)); wT = kb^T inv^T ; attnT = triu(kh qraw^T)
  Scan: u = inv vb - w S ; out = s_q (qraw S + attn u) ; S += kh^T u.
"""
import numpy as np

import concourse.bass as bass
import concourse.mybir as mybir
import concourse.tile as tile
from concourse import bacc
from concourse.bass_utils import run_bass_kernel_spmd
from concourse.masks import make_identity, make_lower_triangular, make_upper_triangular

B, H, L, D = 4, 8, 4096, 128
C = 128
NT = L // C
NSEQ = (B * H) // 8   # sequences per core
FP = mybir.dt.float32
FR = mybir.dt.float32r
BF = mybir.dt.bfloat16
EPS = 1e-6
AF = mybir.ActivationFunctionType
ALU = mybir.AluOpType
NLEV = 6                 # R-chain top level (exponents up to 2^NLEV)
ACT_LEVELS = (2, 4, 6)   # R levels drained via PE I-accumulate + Act copy
G = 4                    # chunks per group


def _emit_group(nc, work, pp, pscan, cst, Sb, bT, grps, g, ogs, nseq):
    """Pass A + scan for chunk group g (chunks 4g..4g+3) of ALL sequences."""
    identB = cst["identB"]
    mmtag = "mm"
    NS = nseq
    wt = lambda s, tg, shape, dt, bufs=2: work.tile(
        shape, dt, tag=f"{tg}{s}", name=f"{tg}{s}_{g}", bufs=bufs)

    def drain(par, dst, src_ps, scale=None):
        """Copy-drain on Act (par 0) or DVE (par 1)."""
        if scale is None:
            if par == 0:
                nc.scalar.copy(dst, src_ps)
            else:
                nc.vector.tensor_copy(dst, src_ps)
        else:
            if par == 0:
                nc.scalar.activation(out=dst, in_=src_ps, func=AF.Copy, scale=scale)
            else:
                nc.vector.tensor_scalar_mul(dst, src_ps, scale)

    # ---- norm: sumsq (Pool), sqrt (Act), recip (DVE) ----
    ss = [wt(s, "ss", [C, 2 * G], FP) for s in range(NS)]
    for s in range(NS):
        qt, kt, vt = grps[s]
        for c in range(G):
            scr = work.tile([C, D], BF, tag="scr", name="scr", bufs=4)
            nc.vector.scalar_tensor_tensor(
                out=scr, in0=kt[:, c, :], scalar=1.0, in1=kt[:, c, :],
                op0=ALU.mult, op1=ALU.mult, accum_out=ss[s][:, c:c + 1])
            scr2 = work.tile([C, D], BF, tag="scr2", name="scr2", bufs=4)
            nc.scalar.activation(out=scr2, in_=qt[:, c, :], func=AF.Square,
                                 accum_out=ss[s][:, G + c:G + c + 1])
    for s in range(NS):
        nc.scalar.activation(out=ss[s], in_=ss[s], func=AF.Sqrt,
                             bias=cst["epsT"][:, 0:1], scale=1.0)
    for s in range(NS):
        nc.vector.reciprocal(out=ss[s], in_=ss[s])
    # ---- kh, vb, kb (Pool) ----
    kh = [wt(s, "kh", [C, G, D], BF) for s in range(NS)]
    vb = [wt(s, "vb", [C, G, D], BF) for s in range(NS)]
    kb = [wt(s, "kb", [C, G, D], BF) for s in range(NS)]
    for s in range(NS):
        qt, kt, vt = grps[s]
        for c in range(G):
            kss_c = ss[s][:, c:c + 1]
            bcol = bT[s][:, G * g + c: G * g + c + 1]
            nc.gpsimd.tensor_scalar_mul(kh[s][:, c, :], kt[:, c, :], kss_c)
            nc.gpsimd.tensor_scalar_mul(vb[s][:, c, :], vt[:, c, :], bcol)
            nc.gpsimd.tensor_scalar(out=kb[s][:, c, :], in0=kt[:, c, :],
                                    scalar1=kss_c, scalar2=bcol,
                                    op0=ALU.mult, op1=ALU.mult)

    # ---- transposes: qT (f32r in), kT, kbT (bf16) ----
    qT, kT, kbT = [], [], []
    for s in range(NS):
        qt, kt, vt = grps[s]
        qT_ps = pp.tile([D, G, C], FP, tag=mmtag, name="qT_ps")
        for c in range(G):
            nc.tensor.matmul(qT_ps[:, c, :], qt[:, c, :],
                             cst["ident"], is_transpose=True)
        t = wt(s, "qT", [D, G, C], BF)
        drain(s % 2, t, qT_ps)
        qT.append(t)
    for s in range(NS):
        kT_ps = pp.tile([D, G, C], BF, tag=mmtag, name="kT_ps")
        for c in range(G):
            nc.tensor.matmul(kT_ps[:, c, :], kh[s][:, c, :], identB, is_transpose=True)
        t = wt(s, "kT", [D, G, C], BF)
        drain((s + 1) % 2, t, kT_ps)
        kT.append(t)
    for s in range(NS):
        kbT_ps = pp.tile([D, G, C], BF, tag=mmtag, name="kbT_ps")
        for c in range(G):
            nc.tensor.matmul(kbT_ps[:, c, :], kb[s][:, c, :], identB, is_transpose=True)
        t = wt(s, "kbT", [D, G, C], BF, bufs=1)
        drain(s % 2, t, kbT_ps)
        kbT.append(t)

    # ---- T = kb kh^T ; P0 ; Q0 = P0^T ----
    P, Q = [], []
    for s in range(NS):
        T_ps = pp.tile([C, G, C], FP, tag=mmtag, name="T_ps")
        for c in range(G):
            nc.tensor.matmul(T_ps[:, c, :], kbT[s][:, c, :], kT[s][:, c, :])
        t = wt(s, "Pz", [C, G, C], BF, bufs=1)
        nc.vector.tensor_mul(t, T_ps, cst["mNSL"])
        P.append(t)
    for s in range(NS):
        Q0_ps = pp.tile([C, G, C], BF, tag=mmtag, name="Q0_ps")
        for c in range(G):
            nc.tensor.matmul(Q0_ps[:, c, :], P[s][:, c, :], identB, is_transpose=True)
        t = wt(s, "Q0", [C, G, C], BF, bufs=1)
        drain((s + 1) % 2, t, Q0_ps)
        Q.append(t)

    # ---- Neumann chain; R1 = I + Q0 + Q1 + Q1 Q0 (fused, no R0) ----
    R = [None] * NS
    for l in range(1, NLEV + 1):
        Pn = []
        for s in range(NS):
            Pn_ps = pp.tile([C, G, C], FP, tag=mmtag, name=f"P{l}_ps")
            for c in range(G):
                nc.tensor.matmul(Pn_ps[:, c, :], Q[s][:, c, :], P[s][:, c, :])
            t = wt(s, f"P{l % 2}", [C, G, C], BF)
            W = C - (1 << l)
            if l >= 3:
                nc.gpsimd.memset(t[:, :, W:], 0.0)
                drain((s + l) % 2, t[:, :, :W], Pn_ps[:, :, :W])
            else:
                drain((s + l) % 2, t, Pn_ps)
            Pn.append(t)
        P = Pn
        if l < NLEV:
            Qn = []
            for s in range(NS):
                Qn_ps = pp.tile([C, G, C], BF, tag=mmtag, name=f"Q{l}_ps")
                for c in range(G):
                    nc.tensor.matmul(Qn_ps[:, c, :], P[s][:, c, :], identB, is_transpose=True)
                t = wt(s, f"Q{1 + l % 2}", [C, G, C], BF)
                Wq = 1 << l
                if l >= 3:
                    nc.gpsimd.memset(t[:, :, :Wq], 0.0)
                    drain((s + l + 1) % 2, t[:, :, Wq:], Qn_ps[:, :, Wq:])
                else:
                    drain((s + l + 1) % 2, t, Qn_ps)
                Qn.append(t)
        for s in range(NS):
            Rn_ps = pp.tile([C, G, C], FP, tag=mmtag, name=f"R{l}_ps")
            rtag = f"R{l % 2}" if l < NLEV else "Rf"
            if l == 1:
                # R1 = I + Q0 + Q1 + Q1 Q0 (PSUM accumulate; +Q0 in the drain)
                nc.tensor.matmul(Rn_ps, identB, cst["identBw"], start=True, stop=False)
                nc.tensor.matmul(Rn_ps, identB, Qn[s], start=False, stop=False)
                for c in range(G):
                    nc.tensor.matmul(Rn_ps[:, c, :], P[s][:, c, :], Q[s][:, c, :],
                                     start=False, stop=(c == G - 1))
                t = wt(s, rtag, [C, G, C], BF, bufs=2)
                nc.vector.tensor_add(t, Rn_ps, Q[s])   # + Q0 (add-drain)
                R[s] = t
            elif (s + l) % 2 == 0:
                nc.tensor.matmul(Rn_ps, identB, R[s], start=True, stop=False)
                for c in range(G):
                    nc.tensor.matmul(Rn_ps[:, c, :], P[s][:, c, :], R[s][:, c, :],
                                     start=False, stop=(c == G - 1))
                t = wt(s, rtag, [C, G, C], BF, bufs=2)
                nc.scalar.copy(t, Rn_ps)             # Act drain (PE did the +R)
                R[s] = t
            else:
                for c in range(G):
                    nc.tensor.matmul(Rn_ps[:, c, :], P[s][:, c, :], R[s][:, c, :])
                t = wt(s, rtag, [C, G, C], BF, bufs=2)
                nc.vector.tensor_add(t, Rn_ps, R[s])  # DVE add-drain
                R[s] = t
        if l < NLEV:
            Q = Qn
    invT = R

    # ---- wTn = -(kb^T invT) ; attnT ----
    wTn, attnT = [], []
    for s in range(NS):
        w_ps = pp.tile([D, G, C], FP, tag=mmtag, name="w_ps")
        for c in range(G):
            nc.tensor.matmul(w_ps[:, c, :], kb[s][:, c, :], invT[s][:, c, :])
        t = wt(s, "wTn", [D, G, C], BF)
        drain(s % 2, t, w_ps, scale=-1.0)
        wTn.append(t)
    for s in range(NS):
        a_ps = pp.tile([C, G, C], FP, tag=mmtag, name="a_ps")
        for c in range(G):
            nc.tensor.matmul(a_ps[:, c, :], kT[s][:, c, :], qT[s][:, c, :])
        t = wt(s, "attnT", [C, G, C], BF)
        nc.vector.tensor_mul(t, a_ps, cst["mUI"])
        attnT.append(t)

    # ---- scan (sequential per sequence; stage-interleaved across seqs) ----
    for c in range(G):
        u, sc3s = [], []
        for s in range(NS):
            St = Sb[s]
            sc3 = pscan.tile([C, 3, D], FP, tag="sc", name="sc3")
            sc3s.append(sc3)
            u_ps = sc3[:, 0, :]
            nc.tensor.matmul(u_ps, invT[s][:, c, :], vb[s][:, c, :], start=True, stop=False)
            nc.tensor.matmul(u_ps, wTn[s][:, c, :], St, start=False, stop=True)
            t = work.tile([C, D], BF, tag=f"u{s}", name=f"u{s}", bufs=2)
            drain(s % 2, t, u_ps)                    # u = inv vb - w S
            u.append(t)
        for s in range(NS):
            St = Sb[s]
            out_ps, sd_ps = sc3s[s][:, 1, :], sc3s[s][:, 2, :]
            nc.tensor.matmul(out_ps, qT[s][:, c, :], St, start=True, stop=False)
            nc.tensor.matmul(out_ps, attnT[s][:, c, :], u[s], start=False, stop=True)
            nc.tensor.matmul(sd_ps, kh[s][:, c, :], u[s])      # kh^T u
            nc.vector.tensor_add(St, St, sd_ps)                # S += kh^T u
            # out = s_q * (qraw S + attn_raw u) — q-norm folded in here
            nc.scalar.activation(out=ogs[s][:, c, :], in_=out_ps,
                                 func=AF.Copy, scale=ss[s][:, G + c:G + c + 1])


def build_nc(nseq=NSEQ, nt=NT, repeat=1):
    assert nt % G == 0
    ll = nt * C
    nc = bacc.Bacc(None, target_bir_lowering=False)
    dram = {
        "q": nc.dram_tensor("q", [nseq, ll, D], FP, kind="ExternalInput"),
        "k": nc.dram_tensor("k", [nseq, ll, D], FP, kind="ExternalInput"),
        "v": nc.dram_tensor("v", [nseq, ll, D], FP, kind="ExternalInput"),
        "beta": nc.dram_tensor("beta", [nseq, ll], FP, kind="ExternalInput"),
        "out": nc.dram_tensor("out", [nseq, ll, D], FP, kind="ExternalOutput"),
    }
    q_d, k_d, v_d, o_d = dram["q"], dram["k"], dram["v"], dram["out"]
    rr4 = lambda ap: ap.rearrange("(four c) d -> c four d", four=G)
    with tile.TileContext(nc) as tc:
        with (
            tc.tile_pool(name="consts", bufs=1) as consts,
            tc.tile_pool(name="persist", bufs=1) as persist,
            tc.tile_pool(name="io", bufs=2) as io,
            tc.tile_pool(name="work", bufs=2) as work,
            tc.tile_pool(name="pp", bufs=5, space="PSUM") as pp,
            tc.tile_pool(name="pscan", bufs=3, space="PSUM") as pscan,
        ):
            ident = consts.tile([128, 128], FP, tag="ident", name="ident")
            identR = consts.tile([128, 128], FR, tag="identR", name="identR")
            identB = consts.tile([128, 128], BF, tag="identB", name="identB")
            identBw = consts.tile([128, G, 128], BF, tag="identBw", name="identBw")
            mNSL = consts.tile([128, G, 128], FP, tag="mNSL", name="mNSL")
            mUI = consts.tile([128, G, 128], FP, tag="mUI", name="mUI")
            epsT = consts.tile([128, 1], FP, tag="epsT", name="epsT")
            zeroB = consts.tile([128, 128], BF, tag="zeroB", name="zeroB")
            make_identity(nc, ident)
            nc.vector.tensor_copy(identR, ident)
            nc.vector.tensor_copy(identB, ident)
            for c in range(G):
                nc.vector.tensor_copy(identBw[:, c, :], ident)
                make_lower_triangular(nc, mNSL[:, c, :], val=-1.0, diag=False)
                make_upper_triangular(nc, mUI[:, c, :], val=1.0, diag=True)
            nc.gpsimd.memset(epsT, EPS)
            nc.gpsimd.memset(zeroB, 0.0)
            cst = dict(ident=ident, identR=identR, identB=identB,
                       identBw=identBw, mNSL=mNSL, mUI=mUI, epsT=epsT)

            Sb, bT = [], []
            for s in range(nseq):
                Sbt = persist.tile([D, D], BF, tag=f"Sb{s}", name=f"Sb{s}")
                nc.vector.tensor_copy(Sbt, zeroB)
                Sb.append(Sbt)
                bseq = persist.tile([nt, C], FP, tag=f"bseq{s}", name=f"bseq{s}")
                nc.sync.dma_start(out=bseq,
                                  in_=dram["beta"][s].rearrange("(n c) -> n c", c=C))
                bt_ps = pp.tile([C, nt], FP, tag="mm", name=f"btps{s}")
                nc.tensor.transpose(bt_ps, bseq, ident[:nt, :nt])
                btile = persist.tile([C, nt], FP, tag=f"bT{s}", name=f"bT{s}")
                nc.vector.tensor_copy(btile, bt_ps)
                bT.append(btile)

            for rep in range(repeat):
                if rep > 0:
                    for s in range(nseq):
                        nc.vector.tensor_copy(Sb[s], zeroB)
                for g in range(nt // G):
                    rows = slice(g * G * C, (g + 1) * G * C)
                    grps, ogs = [], []
                    for s in range(nseq):
                        qt = io.tile([C, G, D], FP, tag=f"qt{s}", name=f"qt{s}")
                        kt = io.tile([C, G, D], FP, tag=f"kt{s}", name=f"kt{s}")
                        vt = io.tile([C, G, D], FP, tag=f"vt{s}", name=f"vt{s}")
                        nc.sync.dma_start(out=qt, in_=rr4(q_d[s, rows, :]))
                        nc.sync.dma_start(out=kt, in_=rr4(k_d[s, rows, :]))
                        nc.sync.dma_start(out=vt, in_=rr4(v_d[s, rows, :]))
                        og = io.tile([C, G, D], FP, tag=f"og{s}", name=f"og{s}")
                        grps.append((qt, kt, vt))
                        ogs.append(og)
                    _emit_group(nc, work, pp, pscan, cst, Sb, bT, grps, g, ogs, nseq)
                    for s in range(nseq):
                        nc.sync.dma_start(out=rr4(o_d[s, rows, :]), in_=ogs[s])
    nc.compile()
    return nc


_NC_CACHE = None


def kernel(q, k, v, beta):
    global _NC_CACHE
    if _NC_CACHE is None:
        _NC_CACHE = build_nc()
    nc = _NC_CACHE
    q = np.ascontiguousarray(np.asarray(q, dtype=np.float32)).reshape(B * H, L, D)
    k = np.ascontiguousarray(np.asarray(k, dtype=np.float32)).reshape(B * H, L, D)
    v = np.ascontiguousarray(np.asarray(v, dtype=np.float32)).reshape(B * H, L, D)
    beta = np.ascontiguousarray(np.asarray(beta, dtype=np.float32)).reshape(B * H, L)
    in_maps = []
    for core in range(8):
        sl = slice(core * NSEQ, (core + 1) * NSEQ)
        in_maps.append({
            "q": np.ascontiguousarray(q[sl]),
            "k": np.ascontiguousarray(k[sl]),
            "v": np.ascontiguousarray(v[sl]),
            "beta": np.ascontiguousarray(beta[sl]),
        })
    res = run_bass_kernel_spmd(nc, in_maps, core_ids=list(range(8)))
    out = np.empty((B * H, L, D), dtype=np.float32)
    for core in range(8):
        out[core * NSEQ:(core + 1) * NSEQ] = res.results[core]["out"]
    return out.reshape(B, H, L, D)
